# revision 13
# baseline (speedup 1.0000x reference)
"""Trainium2 Bass kernel for WeightedSignedConv (first_aggr=True) GCN block.

Strategy (8 NeuronCores, one SPMD program):
  - 50000 dst nodes are padded to 50176 = 392 tiles of 128; tiles are
    sorted by edge count and dealt to (core, slot) so all 8 cores see
    nearly identical work per slot (one shared program fits all cores).
  - Host-side: edges are bucketed by dst chunk (2 tiles); both signs share
    one bucket — the one-hot column index carries the sign as a +w offset
    into a [pos | neg] PSUM window, so metadata/iota run in fp16 (integers
    exact to 2048). The 1/count normalization is folded into weights.
  - The gather uses int16 indices, so rows are fetched as PAIRS
    (idx = src>>1, one 512B fp16 descriptor). Within a bucket, edges
    sharing a pair share one gathered row ("layers" give rows to pairs
    with several same-parity edges), cutting SWDGE descriptors ~7%.
    Rows are ordered [parity0-only | both | parity1-only] and the
    p0/p1 block boundaries are min/max across cores so one program
    fits all 8 cores; wrong-parity row slots are masked via w=0.
  - Device-side per core: gather pair rows via gpsimd.dma_gather, build
    S[e, c] = w'_e * (c == dloc_e) with one fused tensor_scalar (fp16),
    accumulate aggT[f, c] += Xg[e, f]^T S[e, c] on the tensor engine in
    one [pos|neg] PSUM window, then project with float32r matmuls
    (1 cyc/row) and finish with fused ReLU+bias on the scalar engine.
  - Output is produced transposed ([256, D_core] per core); the host
    transposes/reorders, which is pure layout assembly.

The kernel is descriptor-generation bound (SWDGE ucode ~7.3ns/row); all
other engines hide underneath the gather.
"""

import numpy as np

P = 128
NCORES = 8
CHUNK_TILES = 2          # dst tiles per PSUM window (256 dsts, 512 S cols)
MSG_DT_NAME = "float16"  # gathered x + S dtype (integers exact to 2048)
PROJ_DT_NAME = "float32r"  # projection matmul dtype
GATHER_MAX = 1024        # max idxs per dma_gather (descriptor ring cap)


def _ceil_div(a, b):
    return (a + b - 1) // b


def _preprocess(x, src, dst, attr, slots_per_core, msg_np):
    """Bucket/pad edges; build per-core device arrays + block metadata."""
    n, f = x.shape
    assert f == P
    tiles_total = NCORES * slots_per_core
    n_pad = tiles_total * P

    pos = attr > 0
    neg = attr < 0
    keep = pos | neg
    absa = np.abs(attr)
    cntp = np.bincount(dst[pos], minlength=n).astype(np.float32)
    cntn = np.bincount(dst[neg], minlength=n).astype(np.float32)
    recp = 1.0 / np.maximum(cntp, 1.0)
    recn = 1.0 / np.maximum(cntn, 1.0)
    w1_all = absa.astype(np.float32) * np.where(pos, recp[dst], recn[dst])

    s_ = src[keep].astype(np.int64)
    d_ = dst[keep].astype(np.int64)
    sg = np.where(pos[keep], 0, 1).astype(np.int64)
    w1 = w1_all[keep].astype(np.float32)
    pairidx = s_ >> 1
    parity = s_ & 1

    tile_g = d_ // P

    # Sorted dealing: tile with edge-count rank r -> core r%8, slot r//8.
    tile_edges = np.bincount(tile_g, minlength=tiles_total)
    rank = np.argsort(np.argsort(-tile_edges))
    tile_core = rank % NCORES
    tile_slot = rank // NCORES

    core = tile_core[tile_g]
    slot = tile_slot[tile_g]
    chunk = slot // CHUNK_TILES
    n_chunks = _ceil_div(slots_per_core, CHUNK_TILES)
    # chunk dst width (in nodes); last chunk may be narrower
    chunk_w = np.minimum(
        (np.arange(n_chunks) + 1) * CHUNK_TILES, slots_per_core
    ) - np.arange(n_chunks) * CHUNK_TILES
    chunk_w = chunk_w * P
    # one-hot column: sign picks the half of the [pos | neg] window
    dloc = (slot % CHUNK_TILES) * P + d_ % P + sg * chunk_w[chunk]

    # ---- dedup: rows are (bucket, pair, layer) ----
    key = core * n_chunks + chunk          # bucket id
    nkeys = NCORES * n_chunks
    # per-edge sequence within (key, pair, parity)
    eorder = np.lexsort((parity, pairidx, key))
    ks = key[eorder]
    ps = pairidx[eorder]
    hs = parity[eorder]
    # group boundaries where (key, pair, parity) changes
    gb = np.ones(ks.size, dtype=bool)
    gb[1:] = (ks[1:] != ks[:-1]) | (ps[1:] != ps[:-1]) | (hs[1:] != hs[:-1])
    gidx = np.cumsum(gb) - 1
    first = np.flatnonzero(gb)
    seq_s = np.arange(ks.size) - first[gidx]

    # per (key, pair): m0, m1
    pb = np.ones(ks.size, dtype=bool)
    pb[1:] = (ks[1:] != ks[:-1]) | (ps[1:] != ps[:-1])
    pgid = np.cumsum(pb) - 1
    npairs_g = pgid[-1] + 1
    m0 = np.bincount(pgid, weights=(hs == 0), minlength=npairs_g).astype(np.int64)
    m1 = np.bincount(pgid, weights=(hs == 1), minlength=npairs_g).astype(np.int64)
    pkey = ks[pb]          # bucket of each (key, pair) group
    ppair = ps[pb]
    L = np.maximum(m0, m1)

    # rows: (pgid, layer l) for l in 0..L-1
    row_pg = np.repeat(np.arange(npairs_g), L)
    row_l = np.arange(row_pg.size) - np.repeat(np.cumsum(L) - L, L)
    rm0 = m0[row_pg]
    rm1 = m1[row_pg]
    # class: 0 = h0-only, 1 = both, 2 = h1-only
    rcls = np.where(row_l < np.minimum(rm0, rm1), 1,
                    np.where(row_l < rm0, 0, 2))
    rkey = pkey[row_pg]
    rpair = ppair[row_pg]

    # order rows within bucket: class 0, 1, 2 (h0 rows contiguous prefix,
    # h1 rows contiguous suffix)
    rorder = np.lexsort((row_l, rpair, rcls, rkey))
    rkey_s = rkey[rorder]
    # rank within bucket
    rfirst = np.searchsorted(rkey_s, np.arange(nkeys), side="left")
    rrank = np.arange(rkey_s.size) - rfirst[np.minimum(rkey_s, nkeys - 1)]
    # per bucket counts
    nrows_b = np.bincount(rkey_s, minlength=nkeys).reshape(NCORES, n_chunks)
    nh0_b = np.bincount(rkey_s, weights=(rcls[rorder] != 2),
                        minlength=nkeys).astype(np.int64).reshape(
                            NCORES, n_chunks)
    nc0_b = np.bincount(rkey_s, weights=(rcls[rorder] == 0),
                        minlength=nkeys).astype(np.int64).reshape(
                            NCORES, n_chunks)

    blocks = np.maximum(_ceil_div(nrows_b.max(axis=0), P), 1)   # [chunk]
    p1 = np.minimum(_ceil_div(nh0_b.max(axis=0), P), blocks)    # h0 in [0,p1)
    p0 = np.minimum(nc0_b.min(axis=0) // P, blocks)             # h1 in [p0,nb)

    # map rows back: global row id (in rorder space) -> (bucket, rank)
    row_rank = np.empty(rkey.size, dtype=np.int64)
    row_rank[rorder] = rrank

    # map edges -> row: edge (pgid, h, seq) -> row (pgid, l=seq)
    # row id in original row space = rowstart[pgid] + seq
    rowstart = np.cumsum(L) - L
    erow = rowstart[pgid] + seq_s          # for edges in eorder space
    edge_rank = row_rank[erow]             # rank within bucket
    # scatter back to original edge order
    erank = np.empty(ks.size, dtype=np.int64)
    erank[eorder] = edge_rank
    eh = parity                            # class of the edge

    # ---- program layout: per chunk, blocks + passes ----
    gstart = np.zeros(n_chunks, dtype=np.int64)
    chunks = []        # (chunk_idx, w, chunk_block0, nb_chunk)
    windows = {}       # chunk -> [(gblock, half, metacol), ...]
    b = 0
    mc = 0
    for c in range(n_chunks):
        gstart[c] = b
        nb = int(blocks[c])
        ops = []
        for j in range(nb):
            if j < p1[c]:
                ops.append((b + j, 0, mc))
                mc += 1
            if j >= p0[c]:
                ops.append((b + j, 1, mc))
                mc += 1
        windows[c] = ops
        b += nb
        chunks.append((c, int(chunk_w[c]), gstart[c], nb,
                       int(nrows_b[:, c].max())))
    tot_blocks = b
    tot_cols = mc
    npad = tot_blocks * P

    # per-row slot in padded per-core arrays
    # rows of bucket (core, chunk) occupy gstart[chunk]*P + rank
    colmap = -np.ones((tot_blocks, 2), dtype=np.int64)
    for ops in windows.values():
        for gbk, h, mcol in ops:
            colmap[gbk, h] = mcol

    idx16_list, dw_list, ww_list = [], [], []
    eslot_all = gstart[chunk] * P + erank
    for cc in range(NCORES):
        # row slots: pairs
        sp = np.zeros(npad, dtype=np.int64)
        msel = (rkey // n_chunks) == cc
        rch = rkey[msel] % n_chunks
        rslot = gstart[rch] * P + row_rank[msel]
        sp[rslot] = rpair[msel]
        tmp = sp.reshape(-1, 16).T.astype(np.int16)
        idx16_list.append(np.tile(tmp, (8, 1)))

        # edge metadata
        me = core == cc
        dcols = np.zeros((P, tot_cols), dtype=np.float64)
        wcols = np.zeros((P, tot_cols), dtype=np.float64)
        es = eslot_all[me]
        ehh = eh[me]
        edl = dloc[me]
        ew = w1[me]
        mcols = colmap[es // P, ehh]
        assert (mcols >= 0).all()
        dcols[es % P, mcols] = edl
        wcols[es % P, mcols] = ew
        dw_list.append(np.ascontiguousarray(dcols).astype(np.float32))
        ww_list.append(np.ascontiguousarray(wcols).astype(np.float32))

    meta = dict(
        n=n,
        n_pad=n_pad,
        slots_per_core=slots_per_core,
        n_chunks=n_chunks,
        tot_blocks=tot_blocks,
        tot_cols=tot_cols,
        npad=npad,
        chunks=chunks,
        windows=windows,
        tile_core=tile_core,
        tile_slot=tile_slot,
    )
    return meta, idx16_list, dw_list, ww_list


def _build_program(meta, msg_dt, proj_dt):
    import concourse.bacc as bacc
    import concourse.mybir as mybir
    import concourse.tile as tile

    f32 = mybir.dt.float32
    dcore = meta["slots_per_core"] * P
    wmax = CHUNK_TILES * P
    npairs = meta["n_pad"] // 2

    nc = bacc.Bacc(
        "TRN2", target_bir_lowering=False, debug=False, num_devices=NCORES,
    )
    xall = nc.dram_tensor("xall", [npairs, 2 * P], msg_dt,
                          kind="ExternalInput")
    idx16 = nc.dram_tensor(
        "idx16", [P, meta["npad"] // 16], mybir.dt.int16, kind="ExternalInput"
    )
    dlocd = nc.dram_tensor(
        "dloc", [P, meta["tot_cols"]], f32, kind="ExternalInput"
    )
    wpd = nc.dram_tensor(
        "wp", [P, meta["tot_cols"]], f32, kind="ExternalInput"
    )
    iotad = nc.dram_tensor("iota", [P, 2 * wmax], msg_dt,
                           kind="ExternalInput")
    xTd = nc.dram_tensor("xT", [P, dcore], proj_dt, kind="ExternalInput")
    wd = {}
    for nm in ("wpl", "wpr", "wnl", "wnr"):
        wd[nm] = nc.dram_tensor(nm, [P, P], proj_dt, kind="ExternalInput")
    bd = {
        0: nc.dram_tensor("bpos", [P, 1], f32, kind="ExternalInput"),
        1: nc.dram_tensor("bneg", [P, 1], f32, kind="ExternalInput"),
    }
    outd = nc.dram_tensor("outT", [2 * P, dcore], f32, kind="ExternalOutput")

    # process chunks largest-first: the tail after the last gather is the
    # last chunk's compute chain, so make that chunk the smallest
    chunk_order = sorted(meta["chunks"], key=lambda c: -c[3])
    nbmax = max(c[3] for c in meta["chunks"])
    # idx columns for the first-processed chunk load in their own DMA so
    # the first gather doesn't wait on the whole index array
    f_cb0, f_nb = chunk_order[0][2], chunk_order[0][3]
    lo_cols, hi_cols = f_cb0 * 8, (f_cb0 + f_nb) * 8

    with tile.TileContext(nc) as tc:
        with tc.tile_pool(name="const", bufs=1) as cpool, \
             tc.tile_pool(name="work", bufs=4) as wpool, \
             tc.tile_pool(name="spool", bufs=8) as spool, \
             tc.tile_pool(name="psum", bufs=2, space="PSUM") as ppool:
            idx_t = cpool.tile([P, meta["npad"] // 16], mybir.dt.int16)
            dloc_t = cpool.tile([P, meta["tot_cols"]], f32)
            wp_t = cpool.tile([P, meta["tot_cols"]], f32)
            iota_t = cpool.tile([P, 2 * wmax], msg_dt)
            w_t = {nm: cpool.tile([P, P], proj_dt, name=f"w_{nm}",
                                  tag=f"w_{nm}") for nm in wd}
            b_t = {s: cpool.tile([P, 1], f32, name=f"b_{s}", tag=f"b_{s}")
                   for s in (0, 1)}
            nc.sync.dma_start(out=idx_t[:, lo_cols:hi_cols],
                              in_=idx16[:, lo_cols:hi_cols])
            if lo_cols > 0:
                nc.sync.dma_start(out=idx_t[:, :lo_cols],
                                  in_=idx16[:, :lo_cols])
            if hi_cols < meta["npad"] // 16:
                nc.sync.dma_start(out=idx_t[:, hi_cols:],
                                  in_=idx16[:, hi_cols:])
            nc.sync.dma_start(out=iota_t[:], in_=iotad[:])
            nc.sync.dma_start(out=dloc_t[:], in_=dlocd[:])
            nc.sync.dma_start(out=wp_t[:], in_=wpd[:])
            for nm in wd:
                nc.sync.dma_start(out=w_t[nm][:], in_=wd[nm][:])
            for s in (0, 1):
                nc.sync.dma_start(out=b_t[s][:], in_=bd[s][:])

            wl = {0: w_t["wpl"], 1: w_t["wnl"]}
            wr = {0: w_t["wpr"], 1: w_t["wnr"]}

            for ci, w, cb0, nb_chunk, nrows_max in chunk_order:
                xg = wpool.tile([P, nb_chunk, 2 * P], msg_dt, name="xg",
                                tag="xg")
                # zero the tail region before gathering: trimmed gathers
                # leave those rows unwritten, and stale SBUF could hold NaN
                # bits (0 * NaN would poison the agg matmul)
                ntz = min(nb_chunk * P, ((nrows_max + 15) // 16) * 16)
                if ntz // P < nb_chunk:
                    nc.vector.memset(xg[:, ntz // P :, :], 0)
                done = 0
                while done < nb_chunk:
                    g = min(nb_chunk - done, GATHER_MAX // P)
                    gb0 = cb0 + done
                    # trim the final call to the real (16-aligned) row count
                    ni = min(g * P, ((nrows_max - done * P + 15) // 16) * 16)
                    ni = max(ni, 16)
                    gi = _ceil_div(ni, P)
                    nc.gpsimd.dma_gather(
                        out_ap=xg[:, done : done + gi, :],
                        in_ap=xall[:],
                        idxs_ap=idx_t[:, gb0 * 8 : gb0 * 8 + _ceil_div(ni, 16)],
                        num_idxs=ni,
                        num_idxs_reg=ni,
                        elem_size=2 * P,
                        single_packet=False,
                    )
                    done += g

                agg_ps = ppool.tile([P, 2 * w], f32, name="agg", tag="agg")
                ops = meta["windows"][ci]
                for j, (gb, h, mcol) in enumerate(ops):
                    s_t = spool.tile([P, 2 * w], msg_dt, name="S", tag="S")
                    nc.vector.tensor_scalar(
                        out=s_t[:],
                        in0=iota_t[:, : 2 * w],
                        scalar1=dloc_t[:, mcol : mcol + 1],
                        scalar2=wp_t[:, mcol : mcol + 1],
                        op0=mybir.AluOpType.is_equal,
                        op1=mybir.AluOpType.mult,
                    )
                    nc.tensor.matmul(
                        out=agg_ps[:],
                        lhsT=xg[:, gb - cb0, h * P : (h + 1) * P],
                        rhs=s_t[:],
                        start=(j == 0),
                        stop=(j == len(ops) - 1),
                    )

                xT_t = wpool.tile([P, w], proj_dt, name="xT", tag="xT")
                nc.sync.dma_start(
                    out=xT_t[:],
                    in_=xTd[:, ci * wmax : ci * wmax + w],
                )
                agg_sb = wpool.tile([P, 2 * w], proj_dt, name="aggsb",
                                    tag="aggsb")
                nc.scalar.copy(out=agg_sb[:], in_=agg_ps[:])
                for s in (0, 1):
                    out_ps = ppool.tile([P, w], f32, name=f"out{s}",
                                        tag=f"out{s}")
                    nc.tensor.matmul(
                        out=out_ps[:], lhsT=wl[s][:],
                        rhs=agg_sb[:, s * w : (s + 1) * w],
                        start=True, stop=False,
                    )
                    nc.tensor.matmul(
                        out=out_ps[:], lhsT=wr[s][:], rhs=xT_t[:],
                        start=False, stop=True,
                    )
                    out_sb = wpool.tile([P, w], f32, name=f"outsb{s}",
                                        tag=f"outsb{s}")
                    nc.scalar.activation(
                        out=out_sb[:], in_=out_ps[:],
                        func=mybir.ActivationFunctionType.Relu,
                        bias=b_t[s][:],
                    )
                    nc.sync.dma_start(
                        out=outd[s * P : (s + 1) * P,
                                 ci * wmax : ci * wmax + w],
                        in_=out_sb[:],
                    )
    nc.compile()
    return nc


def _run(x, edge_index, edge_attr, w_pos_l, w_pos_r, b_pos_r, w_neg_l,
         w_neg_r, b_neg_r, slots_per_core=49, sim=False, trace=False,
         trace_all=False):
    import concourse.mybir as mybir
    from concourse.bass_utils import run_bass_kernel_spmd

    msg_dt = getattr(mybir.dt, MSG_DT_NAME)
    proj_dt = getattr(mybir.dt, PROJ_DT_NAME)
    msg_np = np.dtype(mybir.dt.np(msg_dt))
    proj_np = np.float32  # float32r is float32 bits

    x = np.asarray(x, dtype=np.float32)
    edge_index = np.asarray(edge_index)
    edge_attr = np.asarray(edge_attr, dtype=np.float32)
    n, f = x.shape
    assert f == P

    meta, idx16_list, dw_list, ww_list = _preprocess(
        x, edge_index[0], edge_index[1], edge_attr, slots_per_core, msg_np
    )
    n_pad = meta["n_pad"]
    dcore = slots_per_core * P
    wmax = CHUNK_TILES * P

    xp = np.zeros((n_pad, P), dtype=np.float32)
    xp[:n] = x
    xall = np.ascontiguousarray(xp.reshape(n_pad // 2, 2 * P)).astype(msg_np)
    iota = np.tile(
        np.arange(2 * wmax, dtype=np.float32)[None, :], (P, 1)
    ).astype(msg_np)

    weights = {
        "wpl": np.ascontiguousarray(np.asarray(w_pos_l, np.float32).T),
        "wpr": np.ascontiguousarray(np.asarray(w_pos_r, np.float32).T),
        "wnl": np.ascontiguousarray(np.asarray(w_neg_l, np.float32).T),
        "wnr": np.ascontiguousarray(np.asarray(w_neg_r, np.float32).T),
    }
    weights = {k: v.astype(proj_np) for k, v in weights.items()}
    bpos = np.asarray(b_pos_r, np.float32).reshape(P, 1)
    bneg = np.asarray(b_neg_r, np.float32).reshape(P, 1)

    nc = _build_program(meta, msg_dt, proj_dt)

    tile_core, tile_slot = meta["tile_core"], meta["tile_slot"]
    xtiles = xp.reshape(-1, P, P)
    in_maps = []
    for c in range(NCORES):
        mytiles = np.zeros((slots_per_core, P, P), dtype=np.float32)
        sel = tile_core == c
        mytiles[tile_slot[sel]] = xtiles[sel]
        xT_c = np.ascontiguousarray(
            mytiles.reshape(dcore, P).T
        ).astype(proj_np)
        in_maps.append(
            dict(
                xall=xall,
                idx16=idx16_list[c], dloc=dw_list[c], wp=ww_list[c],
                iota=iota, xT=xT_c,
                bpos=bpos, bneg=bneg, **weights,
            )
        )

    if sim:
        from concourse.bass_interp import MultiCoreSim

        ms = MultiCoreSim(nc, num_cores=NCORES)
        for c in range(NCORES):
            for name, arr in in_maps[c].items():
                ms.cores[c].tensor(name)[:] = arr
        ms.simulate()
        results = [
            {"outT": np.array(ms.cores[c].tensor("outT"))}
            for c in range(NCORES)
        ]
        exec_ns = None
    else:
        br = run_bass_kernel_spmd(
            nc, in_maps, list(range(NCORES)), trace=trace,
            trace_cores=list(range(NCORES)) if (trace and trace_all) else None,
        )
        results = br.results
        exec_ns = br.exec_time_ns

    out = np.empty((n_pad, 2 * P), dtype=np.float32)
    for c in range(NCORES):
        o = results[c]["outT"].T.reshape(slots_per_core, P, 2 * P)
        for k in range(slots_per_core):
            g = np.nonzero((tile_core == c) & (tile_slot == k))[0]
            if g.size:
                out[g[0] * P : g[0] * P + P] = o[k]
    return np.ascontiguousarray(out[:n]), exec_ns


def kernel(**inputs):
    out, _ = _run(**inputs)
    return out
